# revision 38
# baseline (speedup 1.0000x reference)
"""DMPNN layer on 8 Trainium2 NeuronCores.

Sharding: edges are assigned to the core that owns their *destination* node
(50000 nodes / 8 cores = 6250 each), so the scatter-sum is core-local and no
collectives are needed.  Within a core, edges are grouped by 128-node
destination block (scatter-sum becomes an accumulating onehot-matmul into one
PSUM tile per block) and split into lo/hi source halves so gather indices fit
in int16 for dma_gather.  The per-(block, region) chunk counts are maxed
across cores so all 8 cores run the same static program (SPMD); per-core
variation is data only (indices / dest_rel / edge_attr, padded with dummies).

Pipeline (per 512-edge super, software-pipelined one super ahead):
  dma_gather 1024 rows/call, 4 SWDGE queues round-robin, <=1 in-flight call
  per queue (2 calls on one ring wedges the ucode; transpose-mode gather
  crashes this runtime -- both are hard constraints).  PE transposes the
  gathered chunks, mm1 (+edge_attr) -> relu (Scalar) -> mm2 -> relu (DVE,
  deserializes the Scalar queue) -> onehot scatter (onehots batched 4/inst
  via broadcast-AP tensor_tensor in bf16).  Node MLP + LayerNorm run
  feature-major in 512-node groups as dest blocks complete: stats via
  ones-column matmuls, rstd via one Abs_reciprocal_sqrt activation, stat
  rows re-broadcast through K=1 matmuls; output is written feature-major
  [D, NPC] and transposed on the host (see assemble()).

Datapath is bf16 (fp32 PSUM accumulation); set BF16 = False for an all-fp32
fallback.  TGATHER is dead (transpose gather crashes on HW); kept as a flag
only to document that finding.
"""

import os

# The bass kernel executes through jax's axon/neuron platform.  A stray
# JAX_PLATFORMS=cpu (commonly set to keep jax off neuronxcc) would hide the
# NeuronCores, so drop it before jax is first imported.
if os.environ.get("JAX_PLATFORMS", "").strip() == "cpu":
    os.environ.pop("JAX_PLATFORMS")

import numpy as np

N_NODES = 50000
N_EDGES = 640000
D = 128          # node feature dim == hidden == output dim
EA = 32          # edge attr dim
NC = 8           # cores
NPC = N_NODES // NC   # nodes per core
BLK = 128        # node block width (scatter psum tile)
NB = (NPC + BLK - 1) // BLK   # 49 blocks per core (last one 106 nodes)
LO = 32768       # int16-addressable row limit for dma_gather
CHUNK = 128      # edge chunk (scatter/matmul granularity)
SUPER = 512      # edge super-chunk (mm1/relu batching)
GB = 1024        # edges per dma_gather call (one full 1024-desc SWDGE ring;
                 # 2048 wedges the ucode ring on HW -- do not raise)
NQ = 4           # SWDGE queues (ucode max 4)
NG = 512         # node-phase chunk width (4 dest blocks per node-MLP group)
EPS = 1e-5

BF16 = True       # bf16 datapath (gather, matmuls); accumulation stays fp32
TGATHER = False   # transpose-mode dma_gather crashes on HW via this runtime

F32 = np.float32


def _np_cdt():
    import ml_dtypes
    return ml_dtypes.bfloat16 if BF16 else np.float32


def _build_schedule(dest: np.ndarray, src: np.ndarray):
    """Group edges by (core, region, block); pad so the chunk structure is
    identical across cores.  Returns shared schedule + per-core data."""
    core = dest // NPC
    block = (dest % NPC) // BLK
    region = (src >= LO).astype(np.int64)

    key = (core * 2 + region) * NB + block
    order = np.argsort(key, kind="stable")
    key_s = key[order]
    cnt = np.bincount(key, minlength=NC * 2 * NB).reshape(NC, 2, NB)

    # shared chunk counts per (region, block): max over cores, >= 1
    n_chunks = np.maximum(1, -(-cnt.max(axis=0) // CHUNK))  # [2, NB]
    # pad each region's total chunks to a SUPER multiple (extra chunks go to
    # the last block; their edges are all dummies)
    for r in range(2):
        total = int(n_chunks[r].sum())
        extra = (-total) % (SUPER // CHUNK)
        n_chunks[r, NB - 1] += extra
    L = n_chunks.sum(axis=1) * CHUNK          # [2] padded edges per region
    L_lo, L_hi = int(L[0]), int(L[1])
    L_tot = L_lo + L_hi

    # padded start offset of each (region, block) group within a core's stream
    pad_start = np.zeros((2, NB), np.int64)
    pad_start[0] = np.concatenate([[0], np.cumsum(n_chunks[0])[:-1]]) * CHUNK
    pad_start[1] = L_lo + np.concatenate([[0], np.cumsum(n_chunks[1])[:-1]]) * CHUNK

    # rank of each edge within its (core, region, block) group
    grp_start = np.zeros(NC * 2 * NB + 1, np.int64)
    np.cumsum(np.bincount(key, minlength=NC * 2 * NB), out=grp_start[1:])
    rank = np.arange(N_EDGES) - grp_start[key_s]

    # position of each (sorted) edge inside its core's padded stream
    r_s = (key_s // NB) % 2
    b_s = key_s % NB
    c_s = key_s // (2 * NB)
    pos = pad_start[r_s, b_s] + rank

    t_chunks = np.concatenate([np.repeat(np.arange(NB), n_chunks[0]),
                               np.repeat(np.arange(NB), n_chunks[1])])
    blk_of_edge = np.repeat(t_chunks, CHUNK)

    per_core = []
    dest_s = dest[order]
    src_s = src[order]
    for c in range(NC):
        m = c_s == c
        p = pos[m]
        src_pad = np.zeros(L_tot, np.int64)
        src_pad[p] = src_s[m]
        # hi-region dummies (still 0) -> row 0 of the hi view
        src_pad[L_lo:][src_pad[L_lo:] == 0] = LO
        dest_rel = np.full(L_tot, -1.0, F32)
        dest_rel[p] = (dest_s[m] % NPC - blk_of_edge[p] * BLK).astype(F32)
        assert dest_rel.max() < BLK and (dest_rel[p] >= 0).all()
        ea_perm = np.full(L_tot, -1, np.int64)
        ea_perm[p] = order[m]   # original edge id per padded slot (-1 = dummy)
        per_core.append(dict(src=src_pad, dest_rel=dest_rel, ea_perm=ea_perm))

    sched = dict(n_chunks=n_chunks, L_lo=L_lo, L_hi=L_hi, L_tot=L_tot,
                 T_tot=L_tot // CHUNK)
    return sched, per_core


def _wrap_idx(idx: np.ndarray) -> np.ndarray:
    """int16 index array -> SBUF layout [128, L/16] (16-partition wrap,
    replicated for the 8 gpsimd cores)."""
    L = idx.shape[0]
    w = idx.reshape(L // 16, 16).T.astype(np.int16)   # [16, L/16]
    return np.tile(w, (8, 1))                          # [128, L/16]


def _build_bass(sched):
    import concourse.bacc as bacc
    import concourse.mybir as mybir
    import concourse.tile as tile

    dt = mybir.dt
    cdt = dt.bfloat16 if BF16 else dt.float32
    L_lo, L_hi, L_tot = sched["L_lo"], sched["L_hi"], sched["L_tot"]
    n_chunks = sched["n_chunks"]
    T_tot = sched["T_tot"]
    skip_bias2 = sched["skip_bias2"]
    skip_biasn = sched["skip_biasn"]
    skip_affine = sched["skip_affine"]

    nc = bacc.Bacc("TRN2", target_bir_lowering=False, debug=False,
                   num_devices=NC, num_swdge_queues=NQ)

    def din(name, shape, d=None):
        return nc.dram_tensor(name, shape, d or cdt, kind="ExternalInput").ap()

    xg = din("xg", [N_NODES, D])
    IDX0 = min(4 * GB, L_lo)
    idx_l0 = din("idx_l0", [128, IDX0 // 16], dt.int16)
    idx_lo = din("idx_lo", [128, (L_lo - IDX0) // 16], dt.int16)
    idx_hi = din("idx_hi", [128, L_hi // 16], dt.int16)
    ea_t = din("ea_t", [EA, L_tot])
    dr_t = din("dr_t", [128, T_tot])  # bf16: dest_rel values -1..127 exact
    xt_loc = din("xt_loc", [D, NPC])
    xt_f32 = din("xt_f32", [D, NPC], dt.float32)
    w1a = din("w1a", [D, D])
    w1b = din("w1b", [EA, D])
    w2 = din("w2", [D, D])
    wna = din("wna", [D, D])
    wnb = din("wnb", [D, D])
    b1 = din("b1", [D, 1], dt.float32)
    b2r = din("b2r", [1, D])
    bnc = din("bnc", [D, 1], dt.float32)
    iota = din("iota", [128, BLK])
    ident_in = din("ident", [128, 128])
    ones_r = din("ones_r", [1, 128])
    ones_c = din("ones_c", [128, 1], dt.float32)
    onep_f = din("onep_f", [1, 128], dt.float32)
    onen_f = din("onen_f", [1, 128], dt.float32)
    gma = din("gma", [D, 1], dt.float32)
    bta = din("bta", [D, 1], dt.float32)
    # feature-major output [D, NPC]; the host transposes it back
    out = nc.dram_tensor("out", [D, NPC], dt.float32,
                         kind="ExternalOutput").ap()

    with tile.TileContext(nc) as tc:
        from contextlib import ExitStack
        ctx = ExitStack()
        with ctx:
            const = ctx.enter_context(tc.tile_pool(name="const", bufs=1))
            gpool = ctx.enter_context(tc.tile_pool(name="gather", bufs=4))
            eapool = ctx.enter_context(tc.tile_pool(name="ea", bufs=4))
            work = ctx.enter_context(tc.tile_pool(name="work", bufs=6))
            ohpool = ctx.enter_context(tc.tile_pool(name="ohp", bufs=6))
            psum = ctx.enter_context(tc.tile_pool(name="psum", bufs=2,
                                                  space="PSUM"))
            psum_agg = ctx.enter_context(tc.tile_pool(name="psum_agg", bufs=2,
                                                      space="PSUM"))
            npool = ctx.enter_context(tc.tile_pool(name="node", bufs=3))

            def load_const(ap, shape, d=None):
                t = const.tile(shape, d or cdt, tag=f"c_{ap.tensor.name}")
                nc.sync.dma_start(out=t[:], in_=ap)
                return t

            # gather-critical consts first so the first dma_gather can
            # issue as early as possible; node-phase consts load last
            il0_s = const.tile([128, IDX0 // 16], dt.int16, tag="c_idxl0")
            nc.gpsimd.dma_start(out=il0_s[:], in_=idx_l0[:])
            il_s = const.tile([128, (L_lo - IDX0) // 16], dt.int16,
                              tag="c_idxlo")
            nc.sync.dma_start(out=il_s[:], in_=idx_lo[:])
            if not TGATHER:
                ident = load_const(ident_in[:], [128, 128])
            w1a_s = load_const(w1a[:], [D, D])
            w1b_s = load_const(w1b[:], [EA, D])
            b1_s = load_const(b1[:], [D, 1], dt.float32)
            iota_s = load_const(iota[:], [128, BLK])
            dr_s = load_const(dr_t[:], [128, T_tot])
            w2_s = load_const(w2[:], [D, D])
            ones_s = load_const(ones_r[:], [1, 128])
            b2r_s = load_const(b2r[:], [1, D])
            ih_s = load_const(idx_hi[:], [128, L_hi // 16], dt.int16)
            wna_s = load_const(wna[:], [D, D])
            wnb_s = load_const(wnb[:], [D, D])
            bnc_s = load_const(bnc[:], [D, 1], dt.float32)
            onec_s = load_const(ones_c[:], [128, 1], dt.float32)
            onep_s = load_const(onep_f[:], [1, 128], dt.float32)
            onen_s = load_const(onen_f[:], [1, 128], dt.float32)
            if not skip_affine:
                gma_s = load_const(gma[:], [D, 1], dt.float32)
                bta_s = load_const(bta[:], [D, 1], dt.float32)
            xt_s = load_const(xt_loc[:], [D, NPC])
            xtf_s = load_const(xt_f32[:], [D, NPC], dt.float32)

            agg_lo = const.tile([D, NB * BLK], cdt, tag="agg_lo")
            agg_hi = const.tile([D, NB * BLK], cdt, tag="agg_hi")

            eps_t = const.tile([1, 1], dt.float32, tag="eps")
            nc.vector.memset(eps_t[:], EPS)

            # ---------------- edge phase ----------------
            blk_of_chunk = np.concatenate(
                [np.repeat(np.arange(NB), n_chunks[0]),
                 np.repeat(np.arange(NB), n_chunks[1])])
            region_chunks = [int(n_chunks[0].sum()), int(n_chunks[1].sum())]

            def node_group(g):
                """node MLP + residual layernorm for node chunk g (feature-
                major, NG nodes at a time; runs once both regions' agg
                columns for these nodes are final)."""
                n_w = min(NG, NPC - g * NG)
                cols = slice(g * NG, g * NG + n_w)
                # shares slots with ps_t: PSUM budget is 8 banks total
                ps_n = psum.tile([128, NG], dt.float32, tag="ps_t")
                nc.tensor.matmul(ps_n[:, :n_w], wna_s[:], xt_s[:, cols],
                                 start=True, stop=False)
                nc.tensor.matmul(ps_n[:, :n_w], wnb_s[:], agg_lo[:, cols],
                                 start=False, stop=False)
                nc.tensor.matmul(ps_n[:, :n_w], wnb_s[:], agg_hi[:, cols],
                                 start=False, stop=True)
                o_sb = npool.tile([128, NG], dt.float32, tag="o_sb")
                nc.scalar.activation(o_sb[:, :n_w], ps_n[:, :n_w],
                                     mybir.ActivationFunctionType.Relu,
                                     bias=0.0 if skip_biasn else bnc_s[:])
                r_sb = npool.tile([128, NG], dt.float32, tag="r_sb")
                nc.vector.tensor_add(r_sb[:, :n_w], o_sb[:, :n_w],
                                     xtf_s[:, cols])
                sq_sb = npool.tile([128, NG], dt.float32, tag="sq_sb")
                nc.scalar.activation(sq_sb[:, :n_w], r_sb[:, :n_w],
                                     mybir.ActivationFunctionType.Square)
                # per-node mean / mean-square rows via ones-matmuls
                ps_mu = psum_agg.tile([1, NG], dt.float32, tag="st")
                nc.tensor.matmul(ps_mu[:, :n_w], onec_s[:], r_sb[:, :n_w],
                                 start=True, stop=True)
                ps_sq = psum_agg.tile([1, NG], dt.float32, tag="st")
                nc.tensor.matmul(ps_sq[:, :n_w], onec_s[:], sq_sb[:, :n_w],
                                 start=True, stop=True)
                mu_sb = npool.tile([1, NG], dt.float32, tag="mu_sb")
                nc.vector.tensor_copy(mu_sb[:, :n_w], ps_mu[:, :n_w])
                mu2_sb = npool.tile([1, NG], dt.float32, tag="mu2_sb")
                nc.scalar.activation(mu2_sb[:, :n_w], mu_sb[:, :n_w],
                                     mybir.ActivationFunctionType.Square)
                var_sb = npool.tile([1, NG], dt.float32, tag="var_sb")
                nc.vector.tensor_sub(var_sb[:, :n_w], ps_sq[:, :n_w],
                                     mu2_sb[:, :n_w])
                rstd_sb = npool.tile([1, NG], dt.float32, tag="rstd_sb")
                nc.scalar.activation(
                    rstd_sb[:, :n_w], var_sb[:, :n_w],
                    mybir.ActivationFunctionType.Abs_reciprocal_sqrt,
                    bias=eps_t[:])
                # broadcast the stat rows across partitions via K=1 matmuls
                ps_mub = psum_agg.tile([128, NG], dt.float32, tag="st")
                nc.tensor.matmul(ps_mub[:, :n_w], onen_s[:], mu_sb[:, :n_w],
                                 start=True, stop=True)
                y1 = npool.tile([128, NG], dt.float32, tag="y1")
                nc.vector.tensor_add(y1[:, :n_w], r_sb[:, :n_w],
                                     ps_mub[:, :n_w])
                ps_rsb = psum_agg.tile([128, NG], dt.float32, tag="st")
                nc.tensor.matmul(ps_rsb[:, :n_w], onep_s[:], rstd_sb[:, :n_w],
                                 start=True, stop=True)
                y = npool.tile([128, NG], dt.float32, tag="y")
                nc.vector.tensor_mul(y[:, :n_w], y1[:, :n_w],
                                     ps_rsb[:, :n_w])
                if not skip_affine:
                    nc.scalar.activation(y[:, :n_w], y[:, :n_w],
                                         mybir.ActivationFunctionType.Copy,
                                         bias=bta_s[:], scale=gma_s[:])
                nc.sync.dma_start(out=out[:, cols], in_=y[:, :n_w])

            t_glob = 0           # global chunk index (dr_t column)
            n_gather = 0         # gather call counter (queue round-robin)
            for r in range(2):
                L_r = region_chunks[r] * CHUNK
                src_ap = xg[:LO, :] if r == 0 else xg[LO:N_NODES, :]
                idx_s = il_s if r == 0 else ih_s
                n_supers = L_r // SUPER
                ps_ag = None
                cur_blk = -1
                chunks_left = 0
                gbuf_by_k = {}
                pend = []    # (xsT, eab) tiles prepped one super ahead

                n_gcalls = (L_r + GB - 1) // GB

                def idx_slice(e0g, n, r=r):
                    if r == 0 and e0g < IDX0:
                        return il0_s[:, e0g // 16:(e0g + n) // 16]
                    off = e0g - (IDX0 if r == 0 else 0)
                    return idx_s[:, off // 16:(off + n) // 16]

                def issue_gather(k, src_ap=src_ap, L_r=L_r,
                                 gbuf_by_k=gbuf_by_k, n_gcalls=n_gcalls):
                    nonlocal n_gather
                    e0g = k * GB
                    g_n = min(GB, L_r - e0g)
                    gbuf = gpool.tile([128, GB // 128, D], cdt, tag="gbuf")
                    if k == n_gcalls - 1 and g_n > 512:
                        # split the region's final call across two queues so
                        # the tail's data lands sooner
                        h = g_n // 2
                        for j, (lo, n) in enumerate([(0, h), (h, g_n - h)]):
                            nc.gpsimd.dma_gather(
                                gbuf[:, lo // 128:(lo + n) // 128, :], src_ap,
                                idx_slice(e0g + lo, n),
                                n, n, D, elem_step=D,
                                queue_num=(n_gather + j) % NQ)
                        n_gather += 2
                    else:
                        nc.gpsimd.dma_gather(
                            gbuf[:, :g_n // 128, :], src_ap,
                            idx_slice(e0g, g_n),
                            g_n, g_n, D, elem_step=D,
                            queue_num=n_gather % NQ)
                        n_gather += 1
                    gbuf_by_k[k] = gbuf

                def prep_super(s, r=r, L_r=L_r, gbuf_by_k=gbuf_by_k,
                               pend=pend):
                    """transpose + PSUM->SBUF copy + ea prefetch for super s
                    (emitted one super ahead so PE never stalls on the DVE
                    copy, and PE fills cross-engine relu gaps with these
                    transposes)."""
                    e0s = s * SUPER
                    p_sn = min(SUPER, L_r - e0s)
                    p_c0 = (e0s % GB) // CHUNK
                    gbuf = gbuf_by_k[e0s // GB]
                    ps_t = psum.tile([128, SUPER], cdt, tag="ps_t")
                    for k in range(p_sn // CHUNK):
                        nc.tensor.transpose(
                            ps_t[:, k * CHUNK:(k + 1) * CHUNK],
                            gbuf[:, p_c0 + k, :], ident[:])
                    xsT_t = work.tile([128, SUPER], cdt, tag="xsT")
                    nc.vector.tensor_copy(xsT_t[:, :p_sn], ps_t[:, :p_sn])
                    eab = eapool.tile([EA, SUPER], cdt, tag="eab")
                    off = (L_lo if r else 0) + e0s
                    nc.sync.dma_start(out=eab[:, :p_sn],
                                      in_=ea_t[:, off:off + p_sn])
                    pend.append((xsT_t, eab))

                for t_r in range(region_chunks[r]):
                    e0 = t_r * CHUNK            # edge offset within region
                    # ---- super-chunk: edge MLP (software-pipelined) ----
                    if e0 % SUPER == 0:
                        s = e0 // SUPER
                        if s == 0:
                            issue_gather(0)
                            prep_super(0)
                        s_n = min(SUPER, L_r - e0)
                        ns = s_n // CHUNK
                        xsT_t, eab = pend.pop(0)
                        ps1 = psum.tile([128, SUPER], dt.float32, tag="ps_m")
                        nc.tensor.matmul(ps1[:, :s_n], w1a_s[:],
                                         xsT_t[:, :s_n],
                                         start=True, stop=False)
                        nc.tensor.matmul(ps1[:, :s_n], w1b_s[:], eab[:, :s_n],
                                         start=False, stop=True)
                        oh4 = ohpool.tile([128, SUPER // CHUNK, BLK], cdt,
                                          tag="oh4")
                        nc.vector.tensor_tensor(
                            oh4[:, :ns, :],
                            iota_s[:].unsqueeze(1).broadcast_to([128, ns, BLK]),
                            dr_s[:, t_glob:t_glob + ns].unsqueeze(2)
                                .broadcast_to([128, ns, BLK]),
                            mybir.AluOpType.is_equal)
                        # prefetch the next super (and its gather) while the
                        # relu of this super runs
                        if s + 1 < n_supers:
                            e1 = (s + 1) * SUPER
                            if e1 % GB == 0:
                                issue_gather(e1 // GB)
                            prep_super(s + 1)
                        h_sb = work.tile([128, SUPER], cdt, tag="h_sb")
                        nc.scalar.activation(h_sb[:, :s_n], ps1[:, :s_n],
                                             mybir.ActivationFunctionType.Relu,
                                             bias=b1_s[:])
                        # layer 2 (edge-major out) + bias
                        ps2 = psum.tile([128, SUPER], dt.float32, tag="ps_m")
                        for k in range(ns):
                            ksl = slice(k * CHUNK, (k + 1) * CHUNK)
                            nc.tensor.matmul(ps2[:, ksl], h_sb[:, ksl], w2_s[:],
                                             start=True, stop=skip_bias2)
                            if not skip_bias2:
                                nc.tensor.matmul(ps2[:, ksl], ones_s[:],
                                                 b2r_s[:], start=False,
                                                 stop=True)
                        eh_sb = work.tile([128, SUPER], cdt, tag="eh_sb")
                        nc.vector.tensor_scalar_max(eh_sb[:, :s_n],
                                                    ps2[:, :s_n], 0.0)
                    # ---- scatter-sum for this chunk ----
                    b = int(blk_of_chunk[t_glob])
                    if b != cur_blk:
                        assert chunks_left == 0
                        cur_blk = b
                        chunks_left = int(n_chunks[r][b])
                        ps_ag = psum_agg.tile([D, BLK], dt.float32, tag="ps_ag")
                    ksl = slice((e0 % SUPER), (e0 % SUPER) + CHUNK)
                    oh = oh4[:, (e0 % SUPER) // CHUNK, :]
                    first = chunks_left == int(n_chunks[r][b])
                    last = chunks_left == 1
                    nc.tensor.matmul(ps_ag[:], eh_sb[:, ksl], oh,
                                     start=first, stop=last)
                    if last:
                        cols = slice(b * BLK, (b + 1) * BLK)
                        if r == 0:
                            nc.vector.tensor_copy(agg_lo[:, cols], ps_ag[:])
                        else:
                            nc.vector.tensor_copy(agg_hi[:, cols], ps_ag[:])
                            if (b + 1) * BLK >= NG * (b * BLK // NG + 1) \
                                    or b == NB - 1:
                                node_group(b * BLK // NG)
                    chunks_left -= 1
                    t_glob += 1

    nc.compile()
    return nc


def _prepare(**inputs):
    x = np.ascontiguousarray(np.asarray(inputs["x"], F32))
    ei = np.asarray(inputs["edge_index"]).astype(np.int64)
    ea = np.ascontiguousarray(np.asarray(inputs["edge_attr"], F32))
    W_e1 = np.asarray(inputs["W_e1"], F32)
    b_e1 = np.asarray(inputs["b_e1"], F32)
    W_e2 = np.asarray(inputs["W_e2"], F32)
    b_e2 = np.asarray(inputs["b_e2"], F32)
    W_n = np.asarray(inputs["W_n"], F32)
    b_n = np.asarray(inputs["b_n"], F32)
    gamma = np.asarray(inputs["gamma"], F32)
    beta = np.asarray(inputs["beta"], F32)

    cnp = _np_cdt()
    dest, src = ei[0], ei[1]
    sched, per_core = _build_schedule(dest, src)
    sched["skip_bias2"] = bool(np.all(b_e2 == 0))
    sched["skip_biasn"] = bool(np.all(b_n == 0))
    sched["skip_affine"] = bool(np.all(gamma == 1) and np.all(beta == 0))
    nc = _build_bass(sched)

    iota = np.tile(np.arange(BLK, dtype=F32), (128, 1)).astype(cnp)
    ones_r = np.ones((1, 128), cnp)
    ones_c = np.full((128, 1), 1.0 / 128.0, F32)
    gma = gamma[:, None].astype(F32)
    bta = beta[:, None].astype(F32)

    ea_z = np.concatenate([ea, np.zeros((1, EA), F32)], axis=0)  # -1 -> zeros
    xgc = x.astype(cnp)

    in_maps = []
    for c in range(NC):
        pc = per_core[c]
        src_pad = pc["src"]
        L_lo = sched["L_lo"]
        IDX0 = min(4 * GB, sched["L_lo"])
        idx_l0 = _wrap_idx(src_pad[:IDX0].astype(np.int16))
        idx_lo = _wrap_idx(src_pad[IDX0:L_lo].astype(np.int16))
        idx_hi = _wrap_idx((src_pad[L_lo:] - LO).astype(np.int16))
        dr_t = pc["dest_rel"].reshape(-1, CHUNK).T.astype(cnp)  # [128, T_tot]
        ea_t = np.ascontiguousarray(ea_z[pc["ea_perm"]].T.astype(cnp))
        xs = x[c * NPC:(c + 1) * NPC]
        in_maps.append({
            "xg": xgc,
            "idx_l0": idx_l0, "idx_lo": idx_lo, "idx_hi": idx_hi,
            "ea_t": ea_t, "dr_t": dr_t,
            "xt_loc": np.ascontiguousarray(xs.T.astype(cnp)),
            "xt_f32": np.ascontiguousarray(xs.T),
            "w1a": np.ascontiguousarray(W_e1[:D].astype(cnp)),
            "w1b": np.ascontiguousarray(W_e1[D:].astype(cnp)),
            "w2": W_e2.astype(cnp),
            "wna": np.ascontiguousarray(W_n[:D].astype(cnp)),
            "wnb": np.ascontiguousarray(W_n[D:].astype(cnp)),
            "b1": b_e1[:, None].copy(),
            "b2r": b_e2[None, :].astype(cnp),
            "bnc": b_n[:, None].copy(),
            "iota": iota, "ident": np.eye(128).astype(cnp),
            "ones_r": ones_r, "ones_c": ones_c,
            "onep_f": np.ones((1, 128), F32),
            "onen_f": np.full((1, 128), -1.0, F32),
            "gma": gma, "bta": bta,
        })
    return nc, in_maps


def assemble(res) -> np.ndarray:
    """Gather per-core feature-major outputs into the full [N, D] array."""
    return np.concatenate(
        [np.asarray(res.results[c]["out"]).T for c in range(NC)], axis=0)


def kernel(**inputs) -> np.ndarray:
    nc, in_maps = _prepare(**inputs)
    from concourse.bass_utils import run_bass_kernel_spmd
    res = run_bass_kernel_spmd(nc, in_maps, list(range(NC)))
    return assemble(res)

